# revision 7
# baseline (speedup 1.0000x reference)
"""Conditional-RBM Gibbs sampler on 8 Trainium2 NeuronCores (Bass/Tile).

Strategy
--------
Data-parallel: batch 8192 is split into 8 shards of 1024, one per core.

jax.random.bernoulli(k, p) == (jax.random.uniform(k, shape, f32) < p) and the
uniforms do not depend on p, so the whole random stream of the reference is
precomputed on host (CPU threefry, bit-identical to the reference) and turned
into logit-space thresholds q = T*logit(u) (f64, rounded once to f32):
    u < sigmoid(z/T)  <=>  z > T*logit(u)
The device kernel is then a fully deterministic Gibbs chain per step t:
    x   = ve @ W + c_mod          (c_mod folded into q_h on host)
    h   = x > q_h[t]
    a   = h @ W.T ; ab = a + b_mod
    ve' = ab > q_v[t]
    dE  = sum_i (2 v - 1)(a + b)_i = s2 * sum_i (ve_prev-0.5)*ab
    s0  = dE > q_s0[t] ; s2' = 4 s0 - 2
(v_t = s0 ? ve2 : 1-ve2 is only materialized at the very end.)

Matmuls run as fp32r (fp22 multiply, fp32 accumulate) with the weights split
hi/lo by mantissa mask so precision is at full-fp32 level while running at
1 cycle/row; the moving operands are exact {0,1} bit tensors.  The dE
partition-reduction runs as a true-fp32 matmul against a constant 2.0 column.
Everything is unit-major [256 units x 1024 batch] per core so no transposes
are needed anywhere in the loop.
"""

import os
import sys
import numpy as np

for _p in ("/opt/trn_rl_repo", "/root/.axon_site/_ro/trn_rl_repo"):
    if os.path.isdir(_p) and _p not in sys.path:
        sys.path.insert(0, _p)

NV = 256
NH = 256
COND_DIM = 16
HWID = 64
B = 8192
NCORES = 8
BS = B // NCORES  # 1024 per core
T_END = 1.0
T_START = 5.0
FALLOFF = 0.1
_MASK = np.uint32(0xFFFFE000)  # keep 11 mantissa bits (fp22-exact)

_BUILD_CACHE = {}


def _split_hi_lo(w):
    """Split f32 array into hi (11-bit mantissa, fp22-exact) + lo (exact rest)."""
    w = np.ascontiguousarray(w, dtype=np.float32)
    hi = (w.view(np.uint32) & _MASK).view(np.float32)
    lo = (w - hi).astype(np.float32)
    return hi, lo


def _pack_unit_major(x):
    """[B, 256] -> [8, 128, 2048]: out[c, p, mc*1024 + n] = x[c*1024 + n, mc*128 + p]."""
    y = np.ascontiguousarray(x, dtype=np.float32).reshape(NCORES, BS, 2, 128)
    return np.ascontiguousarray(y.transpose(0, 3, 2, 1)).reshape(NCORES, 128, 2048)


def _pack_lhst(w):
    """W [256(k), 256(m)] (hi/lo split) -> [128, 1024] lhsT tiles.

    out[p, ((hl*2+kc)*2+mc)*128 + j] = W_hl[kc*128 + p, mc*128 + j]
    """
    out = np.empty((128, 1024), dtype=np.float32)
    hi, lo = _split_hi_lo(w)
    for hl, wh in enumerate((hi, lo)):
        t = wh.reshape(2, 128, 2, 128)  # [kc, p, mc, j]
        for kc in range(2):
            for mc in range(2):
                out[:, ((hl * 2 + kc) * 2 + mc) * 128:((hl * 2 + kc) * 2 + mc + 1) * 128] = t[kc, :, mc, :]
    return out


def _build(steps):
    if steps in _BUILD_CACHE:
        return _BUILD_CACHE[steps]
    import concourse.mybir as mybir
    from concourse import bacc
    from concourse.tile import TileContext

    f32 = mybir.dt.float32
    f32r = mybir.dt.float32r
    GT = mybir.AluOpType.is_gt
    ADD = mybir.AluOpType.add
    MUL = mybir.AluOpType.mult
    COPY = mybir.ActivationFunctionType.Copy

    nc = bacc.Bacc("TRN2", target_bir_lowering=False, debug=False)

    qh_d = nc.dram_tensor("qh", [steps, 128, 2048], f32, kind="ExternalInput")
    qv_d = nc.dram_tensor("qv", [steps, 128, 2048], f32, kind="ExternalInput")
    qs_d = nc.dram_tensor("qs", [steps, 1, 1024], f32, kind="ExternalInput")
    w1_d = nc.dram_tensor("w1", [128, 1024], f32r, kind="ExternalInput")
    w2_d = nc.dram_tensor("w2", [128, 1024], f32r, kind="ExternalInput")
    bm_d = nc.dram_tensor("bm", [128, 2048], f32, kind="ExternalInput")
    v0_d = nc.dram_tensor("v0", [128, 2048], f32r, kind="ExternalInput")
    vo_d = nc.dram_tensor("vo", [128, 2048], f32, kind="ExternalOutput")

    with TileContext(nc) as tc:
        with tc.tile_pool(name="const", bufs=1) as cpool, \
             tc.tile_pool(name="qst", bufs=2) as qpool, \
             tc.tile_pool(name="vep", bufs=3) as vepool, \
             tc.tile_pool(name="hp", bufs=2) as hpool, \
             tc.tile_pool(name="wk", bufs=2) as wpool, \
             tc.tile_pool(name="row", bufs=2) as rpool, \
             tc.tile_pool(name="mm", bufs=6, space="PSUM") as mpool, \
             tc.tile_pool(name="red", bufs=2, space="PSUM") as dpool:

            w1_sb = cpool.tile([128, 1024], f32r, tag="w1")
            w2_sb = cpool.tile([128, 1024], f32r, tag="w2")
            bm_sb = cpool.tile([128, 2048], f32, tag="bm")
            twos = cpool.tile([128, 1], f32, tag="twos")
            halfL = cpool.tile([1, 128], f32, tag="halfL")
            nc.sync.dma_start(w1_sb[:], w1_d[:])
            nc.sync.dma_start(w2_sb[:], w2_d[:])
            nc.sync.dma_start(bm_sb[:], bm_d[:])
            nc.vector.memset(twos[:], 1.0)
            nc.vector.memset(halfL[:], 0.5)

            ve_prev = vepool.tile([128, 2048], f32r, tag="ve")
            nc.sync.dma_start(ve_prev[:], v0_d[:])
            s2_prev = rpool.tile([1, 1024], f32, tag="s2")
            nc.vector.memset(s2_prev[:], 2.0)

            for t in range(steps):
                qh_t = qpool.tile([128, 2048], f32, tag="qh")
                qv_t = qpool.tile([128, 2048], f32, tag="qv")
                qs_t = rpool.tile([1, 1024], f32, tag="qs")
                nc.sync.dma_start(qh_t[:], qh_d[t])
                nc.sync.dma_start(qv_t[:], qv_d[t])
                nc.sync.dma_start(qs_t[:], qs_d[t])

                h_t = hpool.tile([128, 2048], f32r, tag="h")
                ve_new = vepool.tile([128, 2048], f32r, tag="ve")
                ab_t = wpool.tile([128, 2048], f32, tag="ab")
                mf_t = wpool.tile([128, 2048], f32, tag="mf")
                tmp_t = wpool.tile([128, 2048], f32, tag="tmp")
                # tmp = ve_prev - 0.5 on ACT (1-input, off the DVE critical path)
                for mc2 in range(2):
                    tsl = slice(mc2 * 1024, mc2 * 1024 + 1024)
                    nc.scalar.activation(tmp_t[:, tsl], ve_prev[:, tsl].bitcast(f32),
                                         COPY, bias=-0.5, scale=1.0)
                dE_t = rpool.tile([1, 1024], f32, tag="dE")
                s0_t = rpool.tile([1, 1024], f32, tag="s0")
                s2_new = rpool.tile([1, 1024], f32, tag="s2")

                # ---- mm1: x = W.T @ ve  (+compare -> h) ----
                for mc in range(2):
                    for j in range(2):
                        sl = slice(mc * 1024 + j * 512, mc * 1024 + j * 512 + 512)
                        px = mpool.tile([128, 512], f32, tag="mm")
                        ki = 0
                        for hl in range(2):
                            for kc in range(2):
                                wt = ((hl * 2 + kc) * 2 + mc) * 128
                                nc.tensor.matmul(
                                    px[:],
                                    w1_sb[:, wt:wt + 128],
                                    ve_prev[:, kc * 1024 + j * 512:kc * 1024 + j * 512 + 512],
                                    start=(ki == 0), stop=(ki == 3))
                                ki += 1
                        nc.vector.tensor_tensor(h_t[:, sl], px[:], qh_t[:, sl], GT)

                # ---- mm2: a = W @ h ; ab = a + b_mod ; ve' = ab > qv ; mf ----
                for mc in range(2):
                    for j in range(2):
                        sl = slice(mc * 1024 + j * 512, mc * 1024 + j * 512 + 512)
                        pa = mpool.tile([128, 512], f32, tag="mm")
                        ki = 0
                        for hl in range(2):
                            for kc in range(2):
                                wt = ((hl * 2 + kc) * 2 + mc) * 128
                                nc.tensor.matmul(
                                    pa[:],
                                    w2_sb[:, wt:wt + 128],
                                    h_t[:, kc * 1024 + j * 512:kc * 1024 + j * 512 + 512],
                                    start=(ki == 0), stop=(ki == 3))
                                ki += 1
                        nc.vector.tensor_tensor(ab_t[:, sl], pa[:], bm_sb[:, sl], ADD)
                        nc.vector.tensor_tensor(ve_new[:, sl], ab_t[:, sl], qv_t[:, sl], GT)
                        # mf = (ve_prev - 0.5) * ab, multiply on gpsimd (SBUF-only)
                        nc.gpsimd.tensor_tensor(mf_t[:, sl], tmp_t[:, sl], ab_t[:, sl], MUL)

                # ---- dE reduce + s0 chain ----
                for j in range(2):
                    rsl = slice(j * 512, j * 512 + 512)
                    pr = dpool.tile([1, 512], f32, tag="red")
                    for kc in range(2):
                        nc.tensor.matmul(
                            pr[:], twos[:],
                            mf_t[:, kc * 1024 + j * 512:kc * 1024 + j * 512 + 512],
                            start=(kc == 0), stop=(kc == 1))
                    nc.vector.tensor_tensor(dE_t[:, rsl], pr[:], s2_prev[:, rsl], MUL)
                    nc.vector.tensor_tensor(
                        s0_t[:, rsl], dE_t[:, rsl],
                        qs_t[:, j * 512:j * 512 + 512], GT)
                    nc.scalar.activation(s2_new[:, rsl], s0_t[:, rsl], COPY,
                                         bias=-2.0, scale=4.0)

                ve_prev = ve_new
                s2_prev = s2_new

            # ---- final: v = s2/2 * (ve - 0.5) + 0.5 ----
            vfin = wpool.tile([128, 2048], f32, tag="ab")
            vout = wpool.tile([128, 2048], f32, tag="mf")
            for j in range(2):
                pb = mpool.tile([128, 512], f32, tag="mm")
                nc.tensor.matmul(pb[:], halfL[:], s2_prev[:, j * 512:j * 512 + 512],
                                 start=True, stop=True)
                for mc in range(2):
                    sl = slice(mc * 1024 + j * 512, mc * 1024 + j * 512 + 512)
                    nc.vector.scalar_tensor_tensor(
                        vfin[:, sl], ve_prev[:, sl].bitcast(f32), -0.5, pb[:],
                        ADD, MUL)
                    nc.scalar.activation(vout[:, sl], vfin[:, sl], COPY,
                                         bias=0.5, scale=1.0)
            nc.sync.dma_start(vo_d[:], vout[:])

    nc.compile()
    _BUILD_CACHE[steps] = nc
    return nc


def _host_precompute(cond, W, b, c, fc1_w, fc1_b, fc2_w, fc2_b, steps):
    """Reference-bit-exact conditioner + RNG stream -> packed device inputs."""
    import jax
    import jax.numpy as jnp

    cpu = jax.devices("cpu")[0]
    with jax.default_device(cpu):
        condj = jnp.asarray(np.asarray(cond, dtype=np.float32))
        Wj = jnp.asarray(np.asarray(W, dtype=np.float32))
        bj = jnp.asarray(np.asarray(b, dtype=np.float32))
        cj = jnp.asarray(np.asarray(c, dtype=np.float32))
        fc1w = jnp.asarray(np.asarray(fc1_w, dtype=np.float32))
        fc1b = jnp.asarray(np.asarray(fc1_b, dtype=np.float32))
        fc2w = jnp.asarray(np.asarray(fc2_w, dtype=np.float32))
        fc2b = jnp.asarray(np.asarray(fc2_b, dtype=np.float32))

        # conditioner, op-for-op as in the reference
        x = jnp.tanh(condj @ fc1w.T + fc1b)
        x = x @ fc2w.T + fc2b
        gamma_b, beta_b, gamma_c, beta_c = jnp.split(x, [NV, 2 * NV, 2 * NV + NH], axis=-1)
        b_mod = (1.0 + gamma_b) * bj + beta_b
        c_mod = (1.0 + gamma_c) * cj + beta_c
        b_mod = np.asarray(b_mod)
        c_mod = np.asarray(c_mod)

        # temperature schedule, as in the reference (f32)
        tt = jnp.arange(steps, dtype=jnp.float32)
        s = 1.0 / (1.0 + jnp.exp(FALLOFF * (tt - steps / 2.0)))
        temps = np.asarray(T_END + (T_START - T_END) * s)

        # RNG stream, same key chain as the reference
        key = jax.random.key(42)
        key, k0 = jax.random.split(key)
        v0 = np.asarray(jax.random.bernoulli(k0, 0.5, (B, NV))).astype(np.float32)

        qh = np.empty((NCORES, steps, 128, 2048), dtype=np.float32)
        qv = np.empty((NCORES, steps, 128, 2048), dtype=np.float32)
        qs = np.empty((NCORES, steps, 1024), dtype=np.float32)
        cm64 = c_mod.astype(np.float64)

        def logit64(u):
            u = np.asarray(u).astype(np.float64)
            return np.log(u) - np.log1p(-u)

        for t in range(steps):
            key, kh, ks0, kv = jax.random.split(key, 4)
            T = np.float64(temps[t])
            u_h = jax.random.uniform(kh, (B, NH), jnp.float32)
            u_s = jax.random.uniform(ks0, (B,), jnp.float32)
            u_v = jax.random.uniform(kv, (B, NV), jnp.float32)
            qh_t = (T * logit64(u_h) - cm64).astype(np.float32)
            qv_t = (T * logit64(u_v)).astype(np.float32)
            qs_t = (T * logit64(u_s)).astype(np.float32)
            qh[:, t] = _pack_unit_major(qh_t)
            qv[:, t] = _pack_unit_major(qv_t)
            qs[:, t] = qs_t.reshape(NCORES, 1024)

    return b_mod, v0, qh, qv, qs, temps


def kernel(cond, W, b, c, fc1_w, fc1_b, fc2_w, fc2_b, steps):
    steps = int(steps)
    b_mod, v0, qh, qv, qs, temps = _host_precompute(
        cond, W, b, c, fc1_w, fc1_b, fc2_w, fc2_b, steps)

    W = np.asarray(W, dtype=np.float32)
    w1 = _pack_lhst(W)                           # mm1 lhsT = W
    w2 = _pack_lhst(np.ascontiguousarray(W.T))   # mm2 lhsT = W.T
    bm = _pack_unit_major(b_mod)
    v0p = _pack_unit_major(v0)

    nc = _build(steps)
    from concourse.bass_utils import run_bass_kernel_spmd

    in_maps = []
    for cix in range(NCORES):
        in_maps.append(dict(
            qh=np.ascontiguousarray(qh[cix]),
            qv=np.ascontiguousarray(qv[cix]),
            qs=np.ascontiguousarray(qs[cix]).reshape(steps, 1, 1024),
            w1=w1, w2=w2,
            bm=np.ascontiguousarray(bm[cix]),
            v0=np.ascontiguousarray(v0p[cix]),
        ))
    res = run_bass_kernel_spmd(nc, in_maps, core_ids=list(range(NCORES)))

    v = np.empty((B, NV), dtype=np.float32)
    for cix in range(NCORES):
        vo = res.results[cix]["vo"].reshape(128, 2, 1024)
        v[cix * BS:(cix + 1) * BS] = vo.transpose(2, 1, 0).reshape(BS, NV)
    return v, temps.astype(np.float32)
